# revision 35
# baseline (speedup 1.0000x reference)
"""Trainium2 Bass kernel for Gumbel 2:4-masked Linear (tensor-parallel over out_features).

Math (matches the reference in forward value):
  idx    = argmax over 6 logits per 4-weight block
           (logits = choice_weights + gumbel_noise; constant choice -> shift
            does not change the argmax, so it is skipped in 'const' mode)
  mask   = MASKING_PATTERNS[idx]          (six 2-of-4 binary patterns)
  out    = x @ (weight * mask).T + bias

Distribution: 8 NeuronCores, sharded by output rows (512 rows/core). Mask
generation and the masked GEMM are fully local; outputs concatenated on host.

Mask math: pattern j is the edge {p,q} of K4 with 1s at positions p,q; S_p =
patterns with a 1 at position p = {3,4,5}/{1,2,5}/{0,2,4}/{0,1,3}.
mask[p] = [max_{j in S_p} g_j == max_j g_j], computed as
  DVE : fused paired group maxes (gumbel host-planarized per chunk with plane
        order [g4,g3,g2,g1,g0,g5] so pairs are ascending strided slices)
  Pool: e_p = M_p - mx   (plain subtract; Pool ALU has no max/compare and
        cannot touch PSUM)
  ACT : col_p = Relu(e_p * 1e14 + 1)  (e_p <= 0, == 0 iff p selected)
Exact except on fp32 ties / |e| < 1e-14, which are measure-zero.

Schedule (per core), engineered against the TimelineSim cost model:
  - GEMM operands bf16 (tolerance 2e-2; halves x/w HBM traffic), psum f32.
  - x host-packed [128 part, tt, kt, 128] so each token-strip load is one
    contiguous 8KB/partition DMA at full DMA efficiency.
  - phase A is DMA/DVE-wall limited (gumbel f32 + weights must land before
    the last mask chunk): 3 warm strips hold psum banks across all 4 chunks
    and 6 windowed strips run two 2-chunk passes accumulated in SBUF (bf16),
    giving PE ~40us of mask-independent work; all DMAs issue from SP in
    need-order (g/w per chunk, x just-in-time).
  - phase B streams the remaining 23 strips over the resident transposed
    masked weight (32 matmuls into one psum bank each, DVE bias-add drain);
    the last strip splits into o-halves so its drain+store overlap matmuls.
"""

import numpy
import numpy as np

N_CORES = 8
T = 4096          # tokens = 2*2048
K = 4096          # in_features
O_FULL = 4096     # out_features
O = O_FULL // N_CORES          # 512 out rows per core
GUM_COLS = K // 4 * 6          # 6144 logit floats per weight row
N_KC = 4                       # k chunks in phase 1
KC_K = K // N_KC               # 1024 k per chunk
KC_B = KC_K // 4               # 256 blocks per chunk row
KC_G = KC_B * 6                # 1536 logit floats per chunk row
N_KT = K // 128                # 32 k-tiles for the GEMM
N_OT = O // 128                # 4 o-tiles per core
N_TT = T // 128                # 32 token strips
N_JT = N_KT // N_KC            # 8 k-tiles per chunk
V = 9                          # windowed strips overlapped with phase 1

_prog_cache = {}


def _build_program(mode):
    """mode: 'const' (constant choice_weights folded away) or 'full'."""
    import concourse.bacc as bacc
    import concourse.bass as bass
    import concourse.mybir as mybir
    import concourse.tile as tile
    from concourse.masks import make_identity

    f32 = mybir.dt.float32
    bf16 = mybir.dt.bfloat16
    Alu = mybir.AluOpType

    nc = bacc.Bacc(trn_type="TRN2")
    xt_d = nc.declare_dram_parameter("xt", [128, N_TT * K], bf16, isOutput=False)
    w_d = nc.declare_dram_parameter("w", [O, K], bf16, isOutput=False)
    b_d = nc.declare_dram_parameter("b", [1, O], f32, isOutput=False)
    g_d = nc.declare_dram_parameter("g", [O, GUM_COLS], f32, isOutput=False)
    if mode == "full":
        cw_d = nc.declare_dram_parameter("cw", [O, GUM_COLS], f32, isOutput=False)
    out_d = nc.declare_dram_parameter("out", [T, O], f32, isOutput=True)
    xt_v = xt_d.rearrange("p (t c) -> p t c", t=N_TT)

    with tile.TileContext(nc) as tc:
        with (
            tc.tile_pool(name="singles", bufs=1) as singles,
            tc.tile_pool(name="wmt", bufs=N_KC) as wmt_pool,
            tc.tile_pool(name="gum", bufs=8) as gum_pool,
            tc.tile_pool(name="wtile", bufs=4) as w_pool,
            tc.tile_pool(name="mtmp", bufs=2) as mtmp,
            tc.tile_pool(name="xt", bufs=9) as xt_pool,
            tc.tile_pool(name="osb", bufs=1) as osb_pool,
            tc.tile_pool(name="outs", bufs=4) as out_pool,
            tc.tile_pool(name="ps_x", bufs=2, space="PSUM") as ps_x,
            tc.tile_pool(name="ps_warm", bufs=3, space="PSUM") as ps_warm,
            tc.tile_pool(name="ps_acc", bufs=3, space="PSUM") as ps_acc,
        ):
            ident_f32 = singles.tile([128, 128], f32)
            make_identity(nc, ident_f32)
            ident = singles.tile([128, 128], bf16, name="ident_bf")
            nc.scalar.copy(ident, ident_f32)
            bias_s = singles.tile([128, O], f32)
            nc.gpsimd.dma_start(
                out=bias_s,
                in_=bass.AP(tensor=b_d, offset=0, ap=[[0, 128], [1, O]]),
            )

            # resident transposed masked weight: wmt[kc][p=k%128, j, o]
            wmt = [
                wmt_pool.tile([128, N_JT, O], bf16, name=f"wmt{i}", tag=f"wmt{i}", bufs=1)
                for i in range(N_KC)
            ]

            g_tiles = {}
            w_tiles = {}

            def g_dma(kc, ot, q):
                rows = slice(ot * 128, (ot + 1) * 128)
                g_t = gum_pool.tile([128, KC_G], f32, tag="gum", name=f"g{kc}{ot}")
                q.dma_start(out=g_t, in_=g_d[rows, kc * KC_G:(kc + 1) * KC_G])
                g_tiles[(kc, ot)] = g_t
                if mode == "full":
                    cw_t = gum_pool.tile([128, KC_G], f32, tag="cw", name=f"c{kc}{ot}")
                    q.dma_start(out=cw_t, in_=cw_d[rows, kc * KC_G:(kc + 1) * KC_G])
                    g_tiles[(kc, ot, "cw")] = cw_t

            def w_dma(kc, ot, q):
                rows = slice(ot * 128, (ot + 1) * 128)
                w_t = w_pool.tile([128, KC_K], bf16, tag="w", name=f"w{kc}{ot}")
                q.dma_start(out=w_t, in_=w_d[rows, kc * KC_K:(kc + 1) * KC_K])
                w_tiles[(kc, ot)] = w_t

            def mask_unit(kc, ot, eng):
                """Masked weight for rows [128*ot, 128*(ot+1)), cols kc-chunk,
                transposed into wmt[kc].

                Host delivers gumbel PLANAR per chunk (6 planes of KC_B) with
                plane order [g4,g3,g2,g1,g0,g5], so fused pair ops (DVE) use
                ascending contiguous-inner slices and plain per-plane ops
                (GpSimd) are fully contiguous. Planes: 0=g4 1=g3 2=g2 3=g1
                4=g0 5=g5.  S_p sets: col0={3,4,5} col1={1,2,5} col2={0,2,4}
                col3={0,1,3} (original indices)."""
                g_t = g_tiles[(kc, ot)]
                if mode == "full":
                    E0 = nc.vector if eng == "v" else nc.gpsimd
                    E0.tensor_add(g_t, g_t, g_tiles[(kc, ot, "cw")])
                gs = g_t.rearrange("p (s b) -> p s b", s=6)
                w_t = w_tiles[(kc, ot)]
                wm = w_pool.tile([128, KC_K], bf16, tag="wm", bufs=2)

                # max/compare ops exist only on DVE (Pool ALU: add/sub/mult);
                # fused paired ops: u2=[M0|M1], u4=[M2|M3]
                u2 = mtmp.tile([128, 2, KC_B], f32, tag="u2v", bufs=2)
                u4 = mtmp.tile([128, 2, KC_B], f32, tag="u4v", bufs=2)
                V = nc.vector
                V.tensor_tensor(u2, gs[:, 1:4:2, :], gs[:, 0:3:2, :], op=Alu.max)
                g5b = gs[:, 5:6, :].broadcast_to([128, 2, KC_B])
                V.tensor_tensor(u2, u2, g5b, op=Alu.max)      # [M0|M1]
                g0b = gs[:, 4:5, :].broadcast_to([128, 2, KC_B])
                V.tensor_tensor(u4, g0b, gs[:, 2:4, :], op=Alu.max)
                V.tensor_tensor(u4, u4, gs[:, 0:2, :], op=Alu.max)  # [M2|M3]
                mx = mtmp.tile([128, KC_B], f32, tag="mxv", bufs=2)
                V.tensor_tensor(mx, u2[:, 0, :], gs[:, 2, :], op=Alu.max)
                V.tensor_tensor(mx, mx, u4[:, 1, :], op=Alu.max)
                # compares off DVE: e_p = M_p - mx  (Pool), then
                # col_p = Relu(e_p * 1e14 + 1)      (ACT) — e_p <= 0, and
                # e_p == 0 iff the argmax pattern covers position p.
                e2 = mtmp.tile([128, 2, KC_B], f32, tag="e2", bufs=2)
                e4 = mtmp.tile([128, 2, KC_B], f32, tag="e4", bufs=2)
                P = nc.gpsimd
                P.tensor_sub(e2[:, 0, :], u2[:, 0, :], mx)
                P.tensor_sub(e2[:, 1, :], u2[:, 1, :], mx)
                P.tensor_sub(e4[:, 0, :], u4[:, 0, :], mx)
                P.tensor_sub(e4[:, 1, :], u4[:, 1, :], mx)
                mask = mtmp.tile([128, KC_K], bf16, tag="maskv", bufs=2)
                mv = mask.rearrange("p (b s) -> p s b", s=4)
                Relu = mybir.ActivationFunctionType.Relu
                nc.scalar.activation(mv[:, 0:2, :], e2, Relu, bias=1.0, scale=1e14)
                nc.scalar.activation(mv[:, 2:4, :], e4, Relu, bias=1.0, scale=1e14)
                MU = nc.gpsimd if eng == "g" else nc.vector
                MU.tensor_mul(wm, w_t, mask)

                ps = ps_x.tile([128, KC_K], bf16, tag="psx")
                for j in range(N_JT):
                    nc.tensor.transpose(
                        ps[:, j * 128:(j + 1) * 128],
                        wm[:, j * 128:(j + 1) * 128],
                        ident,
                    )
                nc.scalar.copy(
                    wmt[kc][:, :, ot * 128:(ot + 1) * 128],
                    ps.rearrange("p (a b) -> p a b", a=N_JT),
                )

            x_tiles = {}

            def load_x(tt):
                xs = xt_pool.tile([128, K], bf16, tag="x")
                nc.sync.dma_start(out=xs, in_=xt_v[:, tt, :])
                x_tiles[tt] = xs.rearrange("p (a b) -> p a b", b=128)

            def pass_burst(tt, kc):
                """One k-chunk's 8 mms for strip tt into a fresh psum tile
                (windowed strips: accumulated into SBUF between passes)."""
                xs3 = x_tiles[tt]
                acc = ps_acc.tile([128, O], f32, tag="acc", name=f"pw{tt}_{kc}")
                for j in range(N_JT):
                    nc.tensor.matmul(
                        acc,
                        xs3[:, kc * N_JT + j, :],
                        wmt[kc][:, j, :],
                        start=(j == 0),
                        stop=(j == N_JT - 1),
                    )
                return acc

            def full_burst(tt, halves=False):
                """All 32 mms for strip tt into one psum accumulation.
                halves=True: two o-half chains so the first half's drain and
                store overlap the second half's matmuls (tail strips)."""
                xs3 = x_tiles[tt]
                acc = ps_acc.tile([128, O], f32, tag="acc", name=f"pf{tt}")
                for osl in ([slice(0, O // 2), slice(O // 2, O)]
                            if halves else [slice(None)]):
                    for kc in range(N_KC):
                        for j in range(N_JT):
                            kt = kc * N_JT + j
                            nc.tensor.matmul(
                                acc[:, osl],
                                xs3[:, kt, :],
                                wmt[kc][:, j, osl],
                                start=(kt == 0),
                                stop=(kt == N_KT - 1),
                            )
                return acc

            def out_dma(tt, o_t):
                nc.sync.dma_start(
                    out=out_d[tt * 128:(tt + 1) * 128, :], in_=o_t
                )

            def drain(tt, acc, eng, split=False):
                # psum reads are DVE-only (GPSIMD cannot access PSUM)
                o_t = out_pool.tile([128, O], f32, tag="o", name=f"o{tt}")
                if split:
                    nc.vector.tensor_add(o_t[:, :O // 2], acc[:, :O // 2],
                                         bias_s[:, :O // 2])
                    nc.vector.tensor_add(o_t[:, O // 2:], acc[:, O // 2:],
                                         bias_s[:, O // 2:])
                else:
                    nc.vector.tensor_add(o_t, acc, bias_s)
                out_dma(tt, o_t)

            engs = ["v", "g"]
            queues = [nc.sync, nc.sync]
            WARM = 3                 # psum-resident strips (kc bursts x4)
            # window strips WARM..V-1: two 2-chunk passes (kc01 -> osb,
            # kc23 -> +bias +osb -> out), so no per-chunk adds serialize the
            # engines against the next chunk's mask units.

            # ---- DMA prefetch stream in need-order -----------------------
            # w(kc) rides right behind g(kc) (the mask mul needs it); x strips
            # interleave just-in-time for warm/window bursts.
            def gw(kc):
                for ot in range(N_OT):
                    g_dma(kc, ot, queues[ot % 2])
                for ot in range(N_OT):
                    w_dma(kc, ot, queues[ot % 2])

            gw(0)
            gw(1)
            for sidx in range(0, 3):
                load_x(sidx)
            gw(2)
            for sidx in range(3, 5):
                load_x(sidx)
            gw(3)
            for sidx in range(5, V):
                load_x(sidx)

            osb = {}
            warm_acc = {}

            def warm_burst(tt, kc):
                xs3 = x_tiles[tt]
                if kc == 0:
                    warm_acc[tt] = ps_warm.tile([128, O], f32, tag="wacc",
                                                name=f"wa{tt}")
                acc = warm_acc[tt]
                for j in range(N_JT):
                    kt = kc * N_JT + j
                    nc.tensor.matmul(
                        acc,
                        xs3[:, kt, :],
                        wmt[kc][:, j, :],
                        start=(kt == 0),
                        stop=(kt == N_KT - 1),
                    )

            def window_pass(i, half):
                """half 0: chunks 0-1 -> osb; half 1: chunks 2-3 -> out."""
                xs3 = x_tiles[i]
                acc = ps_acc.tile([128, O], f32, tag="acc", name=f"pw{i}_{half}")
                for n in range(2 * N_JT):
                    kc = half * 2 + n // N_JT
                    j = n % N_JT
                    nc.tensor.matmul(
                        acc,
                        xs3[:, kc * N_JT + j, :],
                        wmt[kc][:, j, :],
                        start=(n == 0),
                        stop=(n == 2 * N_JT - 1),
                    )
                if half == 0:
                    osb[i] = osb_pool.tile([128, O], bf16, name=f"osb{i}",
                                           tag=f"osb{i}")
                    nc.scalar.copy(osb[i], acc)
                else:
                    o_t = out_pool.tile([128, O], f32, tag="o", name=f"ow{i}")
                    nc.vector.tensor_add(o_t, acc, bias_s)
                    nc.gpsimd.tensor_add(o_t, o_t, osb[i])
                    out_dma(i, o_t)

            # ---- phase A ------------------------------------------------
            for ot in range(N_OT):
                mask_unit(0, ot, engs[ot % 2])
            for ot in range(N_OT):
                mask_unit(1, ot, engs[(1 + ot) % 2])
            for s in range(WARM):
                warm_burst(s, 0)
            for ot in range(N_OT):
                mask_unit(2, ot, engs[ot % 2])
            for s in range(WARM):
                warm_burst(s, 1)
            for i in range(WARM, V):
                window_pass(i, 0)
            for ot in range(N_OT):
                mask_unit(3, ot, engs[(1 + ot) % 2])
            for s in range(WARM):
                warm_burst(s, 2)
            for s in range(WARM):
                warm_burst(s, 3)
            for i in range(WARM, V):
                window_pass(i, 1)
            for s in range(WARM):
                drain(s, warm_acc[s], engs[s % 2])

            # ---- phase B: stream remaining strips over resident wmt ------
            for tt in range(V, N_TT):
                load_x(tt)
                if tt == N_TT - 1:
                    acc = full_burst(tt, halves=True)
                    o_t = out_pool.tile([128, O], f32, tag="o", name="olast")
                    H = O // 2
                    nc.vector.tensor_add(o_t[:, :H], acc[:, :H], bias_s[:, :H])
                    nc.sync.dma_start(out=out_d[tt * 128:(tt + 1) * 128, :H],
                                       in_=o_t[:, :H])
                    nc.vector.tensor_add(o_t[:, H:], acc[:, H:], bias_s[:, H:])
                    nc.sync.dma_start(out=out_d[tt * 128:(tt + 1) * 128, H:],
                                      in_=o_t[:, H:])
                else:
                    acc = full_burst(tt)
                    drain(tt, acc, engs[tt % 2])

    nc.compile()
    return nc


def _get_program(mode, const_c=None):
    if mode not in _prog_cache:
        _prog_cache[mode] = _build_program(mode)
    return _prog_cache[mode]


def pack_inputs(x, weight, bias, choice_weights, gumbel_noise):
    """Host-side prep: returns (mode, per-core input maps)."""
    from concourse import mybir

    bf16 = mybir.dt.np(mybir.dt.bfloat16)

    x = np.asarray(x, dtype=np.float32).reshape(T, K)
    # [tt, i, a, p] -> [p, tt, a, i]: each strip is contiguous per partition
    xp = x.reshape(N_TT, 128, N_KT, 128).transpose(3, 0, 2, 1)
    xt = np.ascontiguousarray(xp).astype(bf16).reshape(128, N_TT * K)
    w = np.asarray(weight, dtype=np.float32).astype(bf16)
    b = np.ascontiguousarray(np.asarray(bias, dtype=np.float32)).reshape(1, O_FULL)
    # device mask math expects planes [g4,g3,g2,g1,g0,g5], planar per chunk:
    # per (row, chunk): [KC_B blocks, 6] -> [6 planes, KC_B]
    PERM = [4, 3, 2, 1, 0, 5]

    def planarize(a):
        a = np.asarray(a, dtype=np.float32)[:, PERM]
        a = a.reshape(O_FULL, N_KC, KC_B, 6).transpose(0, 1, 3, 2)
        return np.ascontiguousarray(a).reshape(O_FULL, GUM_COLS)

    cw = np.asarray(choice_weights, dtype=np.float32)
    g = planarize(gumbel_noise)

    is_const = bool((cw == cw.flat[0]).all())
    mode = "const" if is_const else "full"

    in_maps = []
    for c in range(N_CORES):
        rows = slice(c * O, (c + 1) * O)
        m = {
            "xt": xt,
            "w": np.ascontiguousarray(w[rows]),
            "b": np.ascontiguousarray(b[:, rows]),
            "g": np.ascontiguousarray(g[rows]),
        }
        if mode == "full":
            m["cw"] = np.ascontiguousarray(planarize(cw)[rows])
        in_maps.append(m)
    return mode, in_maps


def kernel(x, weight, bias, choice_weights, gumbel_noise):
    from concourse.bass_utils import run_bass_kernel_spmd

    mode, in_maps = pack_inputs(x, weight, bias, choice_weights, gumbel_noise)
    nc = _get_program(mode)
    res = run_bass_kernel_spmd(nc, in_maps, list(range(N_CORES)))
    parts = [res.results[c]["out"] for c in range(N_CORES)]
    out = np.concatenate(parts, axis=1)  # [T, O_FULL]
    return out.reshape(2, 2048, O_FULL)
